# revision 13
# baseline (speedup 1.0000x reference)
"""Trainium2 Bass kernel for BinaryConv (XNOR-style binarized 3x3 conv).

Reference computation:
    bw  = sign(w) * mean(|w|)                       # [O=256, I=256, 3, 3]
    out = conv2d(x, bw, stride=1, pad=1)            # x: [16, 256, 56, 56]

Strategy: data-parallel over batch across 8 NeuronCores (2 images/core),
binarized weight replicated.  Host computes bw (cheap); the general path
does the conv as 9 shifted matmuls (taps) over channel tiles in PSUM.

Fast path (bw == constant c, the case for torch.rand()*0.01 init): every
output channel equals c * boxsum3x3(channel_sum(x)), so the device
computes one channel per image and the host broadcasts on unshard.

Fast-path v2 pipeline (all knobs cost-model tuned):
  - x is loaded UNPADDED and flat: per (img, channel-half, row-region)
    one SWDGE DMA casts fp32->bf16 in flight into its own tile (no
    accum_op, so the pieces have no inter-DMA dependencies and the DMA
    engines never wait on semaphores).  Regions are sized so compute
    starts as early as possible and the last-landing piece is small.
  - s = x0 + x1 (channel-half add) is fused with the kh fold on
    DVE/ACT/Pool: s has zeroed borders (device memsets), then the paired
    scheme folds kh at 1.5 adds/row (p[j] = s[2j]+s[2j+1], then E/O).
  - PE does the kw fold as 3 tap matmuls per 8-row chunk with a
    stationary c*ones bf16 weight (built on device: memset ones *
    runtime cs), accumulating in fp32 PSUM.  The last chunks of the
    last image are kw-prefolded on DVE into single-tap matmuls.
  - A dummy-matmul chain (on the ones tile, available ~300ns) holds the
    PE p-state at full clock from the very start and fills PE stalls.
  - Evicts copy PSUM partition 0 to an SBUF out tile (ACT/DVE), and
    plain f32 stores go out on the sync HWDGE queue, split so the final
    store covers only the last rows.
"""

import os

import numpy as np

import concourse.bass as bass
import concourse.mybir as mybir
import concourse.tile as tile
from concourse import bacc
from concourse.bass_utils import run_bass_kernel_spmd

# Problem constants (hardcoded per harness contract)
N_FULL, C, H, W = 16, 256, 56, 56
O = 256
KH = KW = 3
N_CORES = 8
N_LOC = N_FULL // N_CORES  # 2 images per core
WP = W + 2  # 58
HP = H + 2  # 58
IT = C // 128  # input-channel tiles
OT = O // 128  # output-channel tiles
HCHUNK = 8  # output rows per PSUM tile -> N = 8*56 = 448 <= 512
NCHUNKS = H // HCHUNK  # 7
NP = HP // 2  # 29 row pairs

F32 = mybir.dt.float32
F32R = mybir.dt.float32r
BF16 = mybir.dt.bfloat16

# Enable jax persistent compilation cache so repeat invocations (and repeat
# processes) skip the minutes-long neuronx-cc compile when possible.
try:
    import jax

    jax.config.update("jax_compilation_cache_dir", "/tmp/jax_comp_cache")
    jax.config.update("jax_persistent_cache_min_compile_time_secs", 0.0)
except Exception:
    pass

_CACHE = {}
LAST_RESULTS = None  # BassKernelResults of the most recent device run


def _new_nc():
    # Bass.__init__ emits four const-pool memsets on gpsimd followed by an
    # all-engine barrier; gpsimd is also the SWDGE load-issue engine, so
    # that preamble sits directly on the load-startup critical path.  This
    # kernel never reads the const tensors and every user op is ordered by
    # its own DMA/compute semaphores, so for the duration of construction
    # route the memsets to DVE (idle at startup) and skip the barrier.
    def memset_on_dve(self, ap, constant):
        return self.bass.vector.memset(ap, constant)

    bass.BassGpSimd.memset = memset_on_dve
    orig_barrier = bass.Bass.all_engine_barrier
    bass.Bass.all_engine_barrier = lambda self, **kw: None
    try:
        return bacc.Bacc(
            "TRN2", target_bir_lowering=False, debug=False, num_devices=N_CORES
        )
    finally:
        del bass.BassGpSimd.memset
        bass.Bass.all_engine_barrier = orig_barrier


def _load_x_tiles(nc, pool, x_d):
    """General path: 4 padded x tiles [128, HP, WP], each one contiguous DMA
    (host pads H and W with zeros)."""
    x_tiles = {}
    for img in range(N_LOC):
        eng = nc.sync if img == 0 else nc.gpsimd
        for it in range(IT):
            xt = pool.tile([128, HP, WP], F32R, name="xt", tag="xt")
            eng.dma_start(xt[:], x_d[img, it * 128 : (it + 1) * 128, :, :])
            x_tiles[(img, it)] = xt
    return x_tiles


def _build_general(reps=1):
    """Full binary conv: out[o] = sum_{i,kh,kw} bw[o,i,kh,kw] * xpad[i,h+kh,w+kw].

    Inputs : x  [N_LOC, C, HP, WP]  (spatially zero-padded on host)
             wt [128, IT*9, O]      (wt[i, it*9+kh*3+kw, o] = bw[o, it*128+i, kh, kw])
    Output : out [N_LOC, O, H, W]
    """
    nc = _new_nc()
    x_d = nc.dram_tensor("x", [N_LOC, C, HP, WP], F32R, kind="ExternalInput").ap()
    wt_d = nc.dram_tensor("wt", [128, IT * 9, O], F32R, kind="ExternalInput").ap()
    out_d = nc.dram_tensor("out", [N_LOC, O, H, W], F32, kind="ExternalOutput").ap()

    with tile.TileContext(nc) as tc:
        with (
            tc.tile_pool(name="xp", bufs=N_LOC * IT) as xp,
            tc.tile_pool(name="wp", bufs=1) as wp,
            tc.tile_pool(name="op", bufs=2) as op,
            tc.tile_pool(name="ps", bufs=8, space=bass.MemorySpace.PSUM) as psp,
        ):
            w_t = wp.tile([128, IT * 9, O], F32R)
            nc.sync.dma_start(w_t[:], wt_d[:])
            for _ in range(reps):
                x_tiles = _load_x_tiles(nc, xp, x_d)
                for img in range(N_LOC):
                    for ot in range(OT):
                        ps_tiles = [
                            psp.tile([128, HCHUNK, W], F32, name="ps", tag="ps")
                            for _ in range(NCHUNKS)
                        ]
                        for it in range(IT):
                            xt = x_tiles[(img, it)]
                            for kh in range(KH):
                                for kw in range(KW):
                                    blk = it * 9 + kh * 3 + kw
                                    lhsT = w_t[:, blk, ot * 128 : (ot + 1) * 128]
                                    for ch in range(NCHUNKS):
                                        h0 = ch * HCHUNK
                                        nc.tensor.matmul(
                                            ps_tiles[ch][:],
                                            lhsT,
                                            xt[
                                                :,
                                                h0 + kh : h0 + kh + HCHUNK,
                                                kw : kw + W,
                                            ],
                                            start=(blk == 0),
                                            stop=(blk == IT * 9 - 1),
                                        )
                        out_t = op.tile([128, H, W], F32)
                        for ch in range(NCHUNKS):
                            nc.vector.tensor_copy(
                                out_t[:, ch * HCHUNK : (ch + 1) * HCHUNK, :],
                                ps_tiles[ch][:],
                            )
                        nc.scalar.dma_start(
                            out_d[img, ot * 128 : (ot + 1) * 128, :, :], out_t[:]
                        )
    nc.compile()
    return nc


def _env_ints(name, default):
    s = os.environ.get(name, default)
    return tuple(int(v) for v in s.split(",")) if s else ()


def _build_fast(reps=1):
    """bw == constant c: out[n,h,w] = c * sum_{i,kh,kw} xpad[n,i,h+kh,w+kw].

    Inputs : x [N_LOC, 2, 128, H, W] fp32 (channel-split view), cs [128,1] (= c)
    Output : out [N_LOC, H, W] fp32
    """
    W0 = int(os.environ.get("BCONV_W0", "52"))
    R0 = _env_ints("BCONV_R0", "17,16,23")  # img0 row-region sizes
    R1 = _env_ints("BCONV_R1", "25,24,7")  # img1 row-region sizes
    # per-(img,region) knobs, flat over img0 regions then img1 regions:
    NREG = len(R0) + len(R1)
    PH = _env_ints("BCONV_PH", "0,0,8,8,6,0")  # half-add rows given to Pool
    PO = _env_ints("BCONV_PO", "0,0,1,1,1,0")  # O-fold ops on Pool (0/1)
    DMS = _env_ints("BCONV_DMS", "0," * (NREG - 1) + "0")  # dummies per region
    assert len(PH) == len(PO) == len(DMS) == NREG
    assert sum(R0) == H and sum(R1) == H

    def regions(sizes):
        out, a = [], 0
        for sz in sizes:
            out.append((a, a + sz))
            a += sz
        return out

    REG = [regions(R0), regions(R1)]

    nc = _new_nc()
    x_d = nc.dram_tensor("x", [N_LOC, 2, 128, H, W], F32, kind="ExternalInput").ap()
    cs_d = nc.dram_tensor("cs", [128, 1], F32, kind="ExternalInput").ap()
    out_d = nc.dram_tensor("out", [N_LOC, H, W], F32, kind="ExternalOutput").ap()

    CH0 = ((0, 8), (8, 16), (16, 24), (24, 32), (32, 40), (40, 48), (48, 56))
    CH1 = ((0, 8), (8, 16), (16, 24), (24, 32), (32, 40), (40, 48), (48, 52), (52, 56))
    CHUNKS = (CH0, CH1)
    # evict groups (uniform chunk sizes within a group; contiguous PSUM slots)
    EG = (((0, 1, 2), (3, 4, 5), (6,)), ((0, 1, 2), (3, 4), (5,), (6,), (7,)))
    # img1 chunk indices computed single-tap via DVE kw-prefold
    PREF = set(_env_ints("BCONV_PREF", ""))

    with tile.TileContext(nc) as tc:
        with (
            tc.tile_pool(name="xp", bufs=1) as xp,
            tc.tile_pool(name="fp", bufs=1) as fpp,
            tc.tile_pool(name="wp", bufs=1) as wp,
            tc.tile_pool(name="op", bufs=1) as op,
            tc.tile_pool(name="ps", bufs=1, space=bass.MemorySpace.PSUM) as psp,
            tc.tile_pool(name="psd", bufs=1, space=bass.MemorySpace.PSUM) as psdp,
        ):
            V, A, G = nc.vector, nc.scalar, nc.gpsimd

            # --- prologue: constants, dummies' weight, xs2 border cols ---
            ones = wp.tile([128, 128], BF16, name="ones", tag="ones")
            V.memset(ones[:], 1.0)
            cs_t = wp.tile([128, 1], F32, name="cs", tag="cs")
            nc.sync.dma_start(cs_t[:], cs_d[:])
            wss = wp.tile([128, 128], BF16, name="wss", tag="wss")
            V.tensor_scalar_mul(wss[:], ones[:], cs_t[:, 0:1])
            psd = psdp.tile([128, 128], F32, name="psd", tag="psd")

            def dummy_mms(n):
                for _ in range(n):
                    nc.tensor.matmul(psd[:], ones[:], ones[:], start=True, stop=True)

            x01s, t_tiles, pt_tiles, xs2_tiles, out_tiles = [], [], [], [], []
            for img in range(N_LOC):
                x01s.append(
                    xp.tile([128, 2, H, W], BF16, name="x01", tag=f"x01_{img}")
                )
                t_tiles.append(fpp.tile([128, H, W], BF16, name="t", tag=f"t{img}"))
                pt_tiles.append(fpp.tile([128, NP, W], BF16, name="pt", tag=f"pt{img}"))
                xs2 = fpp.tile([128, H, WP], BF16, name="xs2", tag=f"xs2{img}")
                V.memset(xs2[:, :, 0:1], 0.0)
                V.memset(xs2[:, :, WP - 1 : WP], 0.0)
                xs2_tiles.append(xs2)
                out_tiles.append(op.tile([1, H, W], F32, name="out", tag=f"out{img}"))
            # one PSUM tile: 7 bank-aligned slots of 512 fp32, sliced per chunk
            ps_all = psp.tile([128, 7, 512], F32, name="ps", tag="ps")

            dummy_mms(W0)

            # --- loads: ONE SWDGE cast-DMA per (img, region), both halves ---
            for img in range(N_LOC):
                for r0, r1 in REG[img]:
                    G.dma_start(
                        x01s[img][:, :, r0:r1, :],
                        x_d[img, :, :, r0:r1, :].transpose([1, 0, 2, 3]),
                    )

            def slot(img, ci):
                return ci % 7

            def emit_chunk_mm(img, ci):
                h0, h1 = CHUNKS[img][ci]
                xs2 = xs2_tiles[img]
                dst = ps_all[:, slot(img, ci), 0 : (h1 - h0) * W]
                if img == N_LOC - 1 and ci in PREF:
                    x3 = fpp.tile([128, h1 - h0, W], BF16, name="x3", tag=f"x3{ci}")
                    V.tensor_add(
                        x3[:], xs2[:, h0:h1, 0:W], xs2[:, h0:h1, 1 : W + 1]
                    )
                    V.tensor_add(x3[:], x3[:], xs2[:, h0:h1, 2 : W + 2])
                    nc.tensor.matmul(dst, wss[:], x3[:], start=True, stop=True)
                else:
                    for kw in range(KW):
                        nc.tensor.matmul(
                            dst,
                            wss[:],
                            xs2[:, h0:h1, kw : kw + W],
                            start=(kw == 0),
                            stop=(kw == KW - 1),
                        )

            def emit_evict(img, grp, eng):
                h0 = CHUNKS[img][grp[0]][0]
                h1 = CHUNKS[img][grp[-1]][1]
                s0 = slot(img, grp[0])
                n = (CHUNKS[img][grp[0]][1] - h0) * W  # uniform chunk size in grp
                src = ps_all[0:1, s0 : s0 + len(grp), 0:n]
                if eng is V:
                    V.tensor_copy(out_tiles[img][:, h0:h1, :], src)
                else:
                    A.copy(out_tiles[img][:, h0:h1, :], src)

            def emit_img(img):
                t, pt, xs2, x01 = (
                    t_tiles[img],
                    pt_tiles[img],
                    xs2_tiles[img],
                    x01s[img],
                )
                pairs_done = 0
                rows_done = 0  # xs2 rows emitted
                mm_done = 0  # chunks emitted
                ev_done = 0  # evict groups emitted
                for ri, (r0, r1) in enumerate(REG[img]):
                    ki = (len(R0) if img else 0) + ri
                    last = ri == len(REG[img]) - 1
                    en = H if last else r1 - 1
                    dummy_mms(DMS[ki])
                    # half-add t rows [r0, r1): Pool takes the last PH rows
                    np_ = min(PH[ki], r1 - r0) if ki < NREG else 0
                    nv = (r1 - r0) - np_
                    if nv > 0:
                        V.tensor_add(
                            t[:, r0 : r0 + nv, :],
                            x01[:, 0, r0 : r0 + nv, :],
                            x01[:, 1, r0 : r0 + nv, :],
                        )
                    if np_ > 0:
                        G.tensor_add(
                            t[:, r0 + nv : r1, :],
                            x01[:, 0, r0 + nv : r1, :],
                            x01[:, 1, r0 + nv : r1, :],
                        )
                    # pairs: p0 = t[0] copy; p[j] = t[2j-1]+t[2j]; p28 = t[55]
                    if ri == 0:
                        V.tensor_copy(pt[:, 0:1, :], t[:, 0:1, :])
                        pairs_done = 1
                    pj = NP - 1 if last else (r1 - 1) // 2 + 1
                    if pj > pairs_done:
                        V.tensor_add(
                            pt[:, pairs_done:pj, :],
                            t[:, 2 * pairs_done - 1 : 2 * pj - 1 : 2, :],
                            t[:, 2 * pairs_done : 2 * pj : 2, :],
                        )
                        pairs_done = pj
                    if last:
                        V.tensor_copy(pt[:, NP - 1 : NP, :], t[:, H - 1 : H, :])
                        pairs_done = NP
                    # E/O folds for xs2 rows [rows_done, en)
                    if en > rows_done:
                        h0, h1 = rows_done, en
                        # E rows (even h): xs2[h] = p[h/2] + t[h+1]
                        e0 = h0 + (h0 % 2)
                        if e0 < h1:
                            ne = (h1 - e0 + 1) // 2
                            V.tensor_add(
                                xs2[:, e0 : e0 + 2 * ne : 2, 1 : W + 1],
                                pt[:, e0 // 2 : e0 // 2 + ne, :],
                                t[:, e0 + 1 : e0 + 2 * ne : 2, :],
                            )
                        # O rows (odd h): xs2[h] = t[h-1] + p[(h+1)/2]
                        o0 = h0 + ((h0 + 1) % 2)
                        if o0 < h1:
                            no = (h1 - o0 + 1) // 2
                            oe = min(o0 + 2 * no, H)
                            eng_o = G if PO[ki] else V
                            eng_o.tensor_add(
                                xs2[:, o0:oe:2, 1 : W + 1],
                                t[:, o0 - 1 : oe - 1 : 2, :],
                                pt[:, (o0 + 1) // 2 : (o0 + 1) // 2 + no, :],
                            )
                        rows_done = en
                    # chunks + evict groups now coverable
                    ch = CHUNKS[img]
                    while mm_done < len(ch) and ch[mm_done][1] <= rows_done:
                        emit_chunk_mm(img, mm_done)
                        mm_done += 1
                    egs = EG[img]
                    while ev_done < len(egs) and egs[ev_done][-1] < mm_done:
                        grp = egs[ev_done]
                        is_tail = img == N_LOC - 1 and ev_done == len(egs) - 1
                        emit_evict(img, grp, V if is_tail else A)
                        ev_done += 1

            for _ in range(reps):
                for img in range(N_LOC):
                    emit_img(img)
                    if img == 0:
                        nc.sync.dma_start(out_d[0], out_tiles[0][0:1, :, :])
                # split final store: early rows as soon as evicted, tail last
                li = N_LOC - 1
                nc.sync.dma_start(out_d[li, 0:48, :], out_tiles[li][0:1, 0:48, :])
                nc.sync.dma_start(out_d[li, 48:56, :], out_tiles[li][0:1, 48:56, :])
    nc.compile()
    return nc


def _get_nc(path, reps=1):
    key = (path, reps)
    nc = _CACHE.get(key)
    if nc is None:
        nc = {"general": _build_general, "fast": _build_fast}[path](reps)
        _CACHE[key] = nc
    return nc


def kernel(x, weight):
    global LAST_RESULTS
    x = np.asarray(x, dtype=np.float32)
    weight = np.asarray(weight, dtype=np.float32)
    assert x.shape == (N_FULL, C, H, W) and weight.shape == (O, C, KH, KW)

    # host-side binarization (tiny): bw = sign(w) * mean(|w|)
    scale = np.mean(np.abs(weight), dtype=np.float32).astype(np.float32)
    bw = np.sign(weight) * scale

    c0 = bw.flat[0]
    use_fast = bool(np.all(bw == c0)) and os.environ.get("BCONV_FORCE_GENERAL") != "1"
    reps = int(os.environ.get("BCONV_REPS", "1"))

    if use_fast:
        # channel-split view [n, half, 128, H, W] (same memory layout)
        x_in = np.ascontiguousarray(x).reshape(N_FULL, 2, 128, H, W)
        nc = _get_nc("fast", reps)
        extra = {"cs": np.full((128, 1), c0, dtype=np.float32)}
    else:
        # zero-pad H and W by 1 on each side (conv padding, done on host)
        x_in = np.zeros((N_FULL, C, HP, WP), dtype=np.float32)
        x_in[:, :, 1 : H + 1, 1 : W + 1] = x
        nc = _get_nc("general", reps)
        # wt[i, it*9 + kh*3 + kw, o] = bw[o, it*128 + i, kh, kw]
        wt = np.ascontiguousarray(
            bw.transpose(1, 2, 3, 0)
            .reshape(IT, 128, KH * KW, O)
            .transpose(1, 0, 2, 3)
            .reshape(128, IT * 9, O)
        )
        extra = {"wt": wt}

    in_maps = [
        {"x": x_in[c * N_LOC : (c + 1) * N_LOC], **extra} for c in range(N_CORES)
    ]
    LAST_RESULTS = run_bass_kernel_spmd(
        nc, in_maps, list(range(N_CORES)), trace=os.environ.get("BCONV_TRACE") == "1"
    )
    if use_fast:
        # device returns one channel per image; broadcast across the 256
        # identical output channels while unsharding
        out = np.empty((N_FULL, O, H, W), dtype=np.float32)
        for c in range(N_CORES):
            out[c * N_LOC : (c + 1) * N_LOC] = LAST_RESULTS.results[c]["out"][
                :, None, :, :
            ]
    else:
        out = np.concatenate(
            [LAST_RESULTS.results[c]["out"] for c in range(N_CORES)], axis=0
        )
    return out


# revision 19
# speedup vs baseline: 1.3231x; 1.3231x over previous
"""Trainium2 Bass kernel for BinaryConv (XNOR-style binarized 3x3 conv).

Reference computation:
    bw  = sign(w) * mean(|w|)                       # [O=256, I=256, 3, 3]
    out = conv2d(x, bw, stride=1, pad=1)            # x: [16, 256, 56, 56]

Strategy: data-parallel over batch across 8 NeuronCores (2 images/core),
binarized weight replicated.  Host computes bw (cheap); the general path
does the conv as 9 shifted matmuls (taps) over channel tiles in PSUM.

Fast path (bw == constant c, the case for torch.rand()*0.01 init): every
output channel equals c * boxsum3x3(channel_sum(x)), so the device
computes one channel per image and the host broadcasts on unshard.

Fast-path v2 pipeline (all knobs cost-model tuned):
  - x is loaded UNPADDED and flat: per (img, channel-half, row-region)
    one SWDGE DMA casts fp32->bf16 in flight into its own tile (no
    accum_op, so the pieces have no inter-DMA dependencies and the DMA
    engines never wait on semaphores).  Regions are sized so compute
    starts as early as possible and the last-landing piece is small.
  - s = x0 + x1 (channel-half add) is fused with the kh fold on
    DVE/ACT/Pool: s has zeroed borders (device memsets), then the paired
    scheme folds kh at 1.5 adds/row (p[j] = s[2j]+s[2j+1], then E/O).
  - PE does the kw fold as 3 tap matmuls per 8-row chunk with a
    stationary c*ones bf16 weight (built on device: memset ones *
    runtime cs), accumulating in fp32 PSUM.  The last chunks of the
    last image are kw-prefolded on DVE into single-tap matmuls.
  - A dummy-matmul chain (on the ones tile, available ~300ns) holds the
    PE p-state at full clock from the very start and fills PE stalls.
  - Evicts copy PSUM partition 0 to an SBUF out tile (ACT/DVE), and
    plain f32 stores go out on the sync HWDGE queue, split so the final
    store covers only the last rows.
"""

import os

import numpy as np

import concourse.bass as bass
import concourse.mybir as mybir
import concourse.tile as tile
from concourse import bacc
from concourse.bass_utils import run_bass_kernel_spmd

# Problem constants (hardcoded per harness contract)
N_FULL, C, H, W = 16, 256, 56, 56
O = 256
KH = KW = 3
N_CORES = 8
N_LOC = N_FULL // N_CORES  # 2 images per core
WP = W + 2  # 58
HP = H + 2  # 58
IT = C // 128  # input-channel tiles
OT = O // 128  # output-channel tiles
HCHUNK = 8  # output rows per PSUM tile -> N = 8*56 = 448 <= 512
NCHUNKS = H // HCHUNK  # 7
NP = HP // 2  # 29 row pairs

F32 = mybir.dt.float32
F32R = mybir.dt.float32r
BF16 = mybir.dt.bfloat16

# Enable jax persistent compilation cache so repeat invocations (and repeat
# processes) skip the minutes-long neuronx-cc compile when possible.
try:
    import jax

    jax.config.update("jax_compilation_cache_dir", "/tmp/jax_comp_cache")
    jax.config.update("jax_persistent_cache_min_compile_time_secs", 0.0)
except Exception:
    pass

_CACHE = {}
LAST_RESULTS = None  # BassKernelResults of the most recent device run


def _new_nc():
    # Bass.__init__ emits four const-pool memsets on gpsimd followed by an
    # all-engine barrier; gpsimd is also the SWDGE load-issue engine, so
    # that preamble sits directly on the load-startup critical path.  This
    # kernel never reads the const tensors and every user op is ordered by
    # its own DMA/compute semaphores, so for the duration of construction
    # route the memsets to DVE (idle at startup) and skip the barrier.
    def memset_on_dve(self, ap, constant):
        return self.bass.vector.memset(ap, constant)

    bass.BassGpSimd.memset = memset_on_dve
    orig_barrier = bass.Bass.all_engine_barrier
    bass.Bass.all_engine_barrier = lambda self, **kw: None
    try:
        return bacc.Bacc(
            "TRN2", target_bir_lowering=False, debug=False, num_devices=N_CORES
        )
    finally:
        del bass.BassGpSimd.memset
        bass.Bass.all_engine_barrier = orig_barrier


def _load_x_tiles(nc, pool, x_d):
    """General path: 4 padded x tiles [128, HP, WP], each one contiguous DMA
    (host pads H and W with zeros)."""
    x_tiles = {}
    for img in range(N_LOC):
        eng = nc.sync if img == 0 else nc.gpsimd
        for it in range(IT):
            xt = pool.tile([128, HP, WP], F32R, name="xt", tag="xt")
            eng.dma_start(xt[:], x_d[img, it * 128 : (it + 1) * 128, :, :])
            x_tiles[(img, it)] = xt
    return x_tiles


def _build_general(reps=1):
    """Full binary conv: out[o] = sum_{i,kh,kw} bw[o,i,kh,kw] * xpad[i,h+kh,w+kw].

    Inputs : x  [N_LOC, C, HP, WP]  (spatially zero-padded on host)
             wt [128, IT*9, O]      (wt[i, it*9+kh*3+kw, o] = bw[o, it*128+i, kh, kw])
    Output : out [N_LOC, O, H, W]
    """
    nc = _new_nc()
    x_d = nc.dram_tensor("x", [N_LOC, C, HP, WP], F32R, kind="ExternalInput").ap()
    wt_d = nc.dram_tensor("wt", [128, IT * 9, O], F32R, kind="ExternalInput").ap()
    out_d = nc.dram_tensor("out", [N_LOC, O, H, W], F32, kind="ExternalOutput").ap()

    with tile.TileContext(nc) as tc:
        with (
            tc.tile_pool(name="xp", bufs=N_LOC * IT) as xp,
            tc.tile_pool(name="wp", bufs=1) as wp,
            tc.tile_pool(name="op", bufs=2) as op,
            tc.tile_pool(name="ps", bufs=8, space=bass.MemorySpace.PSUM) as psp,
        ):
            w_t = wp.tile([128, IT * 9, O], F32R)
            nc.sync.dma_start(w_t[:], wt_d[:])
            for _ in range(reps):
                x_tiles = _load_x_tiles(nc, xp, x_d)
                for img in range(N_LOC):
                    for ot in range(OT):
                        ps_tiles = [
                            psp.tile([128, HCHUNK, W], F32, name="ps", tag="ps")
                            for _ in range(NCHUNKS)
                        ]
                        for it in range(IT):
                            xt = x_tiles[(img, it)]
                            for kh in range(KH):
                                for kw in range(KW):
                                    blk = it * 9 + kh * 3 + kw
                                    lhsT = w_t[:, blk, ot * 128 : (ot + 1) * 128]
                                    for ch in range(NCHUNKS):
                                        h0 = ch * HCHUNK
                                        nc.tensor.matmul(
                                            ps_tiles[ch][:],
                                            lhsT,
                                            xt[
                                                :,
                                                h0 + kh : h0 + kh + HCHUNK,
                                                kw : kw + W,
                                            ],
                                            start=(blk == 0),
                                            stop=(blk == IT * 9 - 1),
                                        )
                        out_t = op.tile([128, H, W], F32)
                        for ch in range(NCHUNKS):
                            nc.vector.tensor_copy(
                                out_t[:, ch * HCHUNK : (ch + 1) * HCHUNK, :],
                                ps_tiles[ch][:],
                            )
                        nc.scalar.dma_start(
                            out_d[img, ot * 128 : (ot + 1) * 128, :, :], out_t[:]
                        )
    nc.compile()
    return nc


def _env_ints(name, default):
    s = os.environ.get(name, default)
    return tuple(int(v) for v in s.split(",")) if s else ()


def _build_fast(reps=1):
    """bw == constant c: out[n,h,w] = c * sum_{i,kh,kw} xpad[n,i,h+kh,w+kw].

    Inputs : x [N_LOC, 2, 128, H, W] fp32 (channel-split view), cs [128,1] (= c)
    Output : out [N_LOC, H, W] fp32
    """
    W0 = int(os.environ.get("BCONV_W0", "52"))
    R0 = _env_ints("BCONV_R0", "17,16,23")  # img0 row-region sizes
    R1 = _env_ints("BCONV_R1", "25,24,7")  # img1 row-region sizes
    # per-(img,region) knobs, flat over img0 regions then img1 regions:
    NREG = len(R0) + len(R1)
    PH = _env_ints("BCONV_PH", "0,0,0,0,0,0")  # half-add rows given to Pool
    PO = _env_ints("BCONV_PO", "0,0,0,0,0,0")  # O-fold ops on Pool (0/1)
    DMS = _env_ints("BCONV_DMS", "0," * (NREG - 1) + "0")  # dummies per region
    assert len(PH) == len(PO) == len(DMS) == NREG
    assert sum(R0) == H and sum(R1) == H

    def regions(sizes):
        out, a = [], 0
        for sz in sizes:
            out.append((a, a + sz))
            a += sz
        return out

    REG = [regions(R0), regions(R1)]

    nc = _new_nc()
    x_d = nc.dram_tensor("x", [N_LOC, 2, 128, H, W], F32, kind="ExternalInput").ap()
    cs_d = nc.dram_tensor("cs", [128, 1], F32, kind="ExternalInput").ap()
    out_d = nc.dram_tensor("out", [N_LOC, H, W], F32, kind="ExternalOutput").ap()

    CH0 = ((0, 8), (8, 16), (16, 24), (24, 32), (32, 40), (40, 48), (48, 56))
    CH1 = ((0, 8), (8, 16), (16, 24), (24, 32), (32, 40), (40, 48), (48, 52), (52, 56))
    CHUNKS = (CH0, CH1)
    # img1 chunk indices computed single-tap via DVE kw-prefold
    PREF = set(_env_ints("BCONV_PREF", ""))

    with tile.TileContext(nc) as tc:
        with (
            tc.tile_pool(name="xp", bufs=1) as xp,
            tc.tile_pool(name="fp", bufs=1) as fpp,
            tc.tile_pool(name="wp", bufs=1) as wp,
            tc.tile_pool(name="op", bufs=1) as op,
            tc.tile_pool(name="ps", bufs=7, space=bass.MemorySpace.PSUM) as psp,
            tc.tile_pool(name="psd", bufs=1, space=bass.MemorySpace.PSUM) as psdp,
        ):
            V, A, G = nc.vector, nc.scalar, nc.gpsimd

            # --- prologue: constants, dummies' weight, xs2 border cols ---
            ones = wp.tile([128, 128], BF16, name="ones", tag="ones")
            V.memset(ones[:], 1.0)
            cs_t = wp.tile([128, 1], F32, name="cs", tag="cs")
            nc.sync.dma_start(cs_t[:], cs_d[:])
            wss = wp.tile([128, 128], BF16, name="wss", tag="wss")
            V.tensor_scalar_mul(wss[:], ones[:], cs_t[:, 0:1])
            psd = psdp.tile([128, 128], F32, name="psd", tag="psd")

            def dummy_mms(n):
                for _ in range(n):
                    nc.tensor.matmul(psd[:], ones[:], ones[:], start=True, stop=True)

            x01s, t_tiles, pt_tiles, xs2_tiles, out_tiles = [], [], [], [], []
            for img in range(N_LOC):
                x01s.append(
                    xp.tile([128, 2, H, W], BF16, name="x01", tag=f"x01_{img}")
                )
                t_tiles.append(fpp.tile([128, H, W], BF16, name="t", tag=f"t{img}"))
                pt_tiles.append(fpp.tile([128, NP, W], BF16, name="pt", tag=f"pt{img}"))
                xs2 = fpp.tile([128, H, WP], BF16, name="xs2", tag=f"xs2{img}")
                V.memset(xs2[:, :, 0:1], 0.0)
                V.memset(xs2[:, :, WP - 1 : WP], 0.0)
                xs2_tiles.append(xs2)
                out_tiles.append(op.tile([1, H, W], F32, name="out", tag=f"out{img}"))

            dummy_mms(W0)

            # --- loads: ONE SWDGE cast-DMA per (img, region), both halves ---
            for img in range(N_LOC):
                for r0, r1 in REG[img]:
                    G.dma_start(
                        x01s[img][:, :, r0:r1, :],
                        x_d[img, :, :, r0:r1, :].transpose([1, 0, 2, 3]),
                    )

            def emit_chunk_mm(img, ci):
                h0, h1 = CHUNKS[img][ci]
                xs2 = xs2_tiles[img]
                ps = psp.tile([128, h1 - h0, W], F32, name="ps", tag="ps")
                if img == N_LOC - 1 and ci in PREF:
                    x3 = fpp.tile([128, h1 - h0, W], BF16, name="x3", tag=f"x3{ci}")
                    V.tensor_add(
                        x3[:], xs2[:, h0:h1, 0:W], xs2[:, h0:h1, 1 : W + 1]
                    )
                    V.tensor_add(x3[:], x3[:], xs2[:, h0:h1, 2 : W + 2])
                    nc.tensor.matmul(ps[:], wss[:], x3[:], start=True, stop=True)
                else:
                    for kw in range(KW):
                        nc.tensor.matmul(
                            ps[:],
                            wss[:],
                            xs2[:, h0:h1, kw : kw + W],
                            start=(kw == 0),
                            stop=(kw == KW - 1),
                        )
                return ps

            def emit_evict(img, ci, ps, eng):
                h0, h1 = CHUNKS[img][ci]
                if eng is V:
                    V.tensor_copy(out_tiles[img][:, h0:h1, :], ps[0:1, :, :])
                else:
                    A.copy(out_tiles[img][:, h0:h1, :], ps[0:1, :, :])

            def emit_img(img):
                t, pt, xs2, x01 = (
                    t_tiles[img],
                    pt_tiles[img],
                    xs2_tiles[img],
                    x01s[img],
                )
                pairs_done = 0
                rows_done = 0  # xs2 rows emitted
                mm_done = 0  # chunks emitted
                for ri, (r0, r1) in enumerate(REG[img]):
                    ki = (len(R0) if img else 0) + ri
                    last = ri == len(REG[img]) - 1
                    en = H if last else r1 - 1
                    dummy_mms(DMS[ki])
                    # half-add t rows [r0, r1): Pool takes the last PH rows
                    np_ = min(PH[ki], r1 - r0) if ki < NREG else 0
                    nv = (r1 - r0) - np_
                    if nv > 0:
                        V.tensor_add(
                            t[:, r0 : r0 + nv, :],
                            x01[:, 0, r0 : r0 + nv, :],
                            x01[:, 1, r0 : r0 + nv, :],
                        )
                    if np_ > 0:
                        G.tensor_add(
                            t[:, r0 + nv : r1, :],
                            x01[:, 0, r0 + nv : r1, :],
                            x01[:, 1, r0 + nv : r1, :],
                        )
                    # pairs: p0 = t[0] copy; p[j] = t[2j-1]+t[2j]; p28 = t[55]
                    if ri == 0:
                        V.tensor_copy(pt[:, 0:1, :], t[:, 0:1, :])
                        pairs_done = 1
                    pj = NP - 1 if last else (r1 - 1) // 2 + 1
                    if pj > pairs_done:
                        V.tensor_add(
                            pt[:, pairs_done:pj, :],
                            t[:, 2 * pairs_done - 1 : 2 * pj - 1 : 2, :],
                            t[:, 2 * pairs_done : 2 * pj : 2, :],
                        )
                        pairs_done = pj
                    if last:
                        V.tensor_copy(pt[:, NP - 1 : NP, :], t[:, H - 1 : H, :])
                        pairs_done = NP
                    # E/O folds for xs2 rows [rows_done, en)
                    if en > rows_done:
                        h0, h1 = rows_done, en
                        # E rows (even h): xs2[h] = p[h/2] + t[h+1]
                        e0 = h0 + (h0 % 2)
                        if e0 < h1:
                            ne = (h1 - e0 + 1) // 2
                            V.tensor_add(
                                xs2[:, e0 : e0 + 2 * ne : 2, 1 : W + 1],
                                pt[:, e0 // 2 : e0 // 2 + ne, :],
                                t[:, e0 + 1 : e0 + 2 * ne : 2, :],
                            )
                        # O rows (odd h): xs2[h] = t[h-1] + p[(h+1)/2]
                        o0 = h0 + ((h0 + 1) % 2)
                        if o0 < h1:
                            no = (h1 - o0 + 1) // 2
                            oe = min(o0 + 2 * no, H)
                            eng_o = G if PO[ki] else V
                            eng_o.tensor_add(
                                xs2[:, o0:oe:2, 1 : W + 1],
                                t[:, o0 - 1 : oe - 1 : 2, :],
                                pt[:, (o0 + 1) // 2 : (o0 + 1) // 2 + no, :],
                            )
                        rows_done = en
                    # chunks now coverable; evict each right after its taps
                    ch = CHUNKS[img]
                    while mm_done < len(ch) and ch[mm_done][1] <= rows_done:
                        ps = emit_chunk_mm(img, mm_done)
                        tail2 = img == N_LOC - 1 and mm_done == len(ch) - 2
                        emit_evict(img, mm_done, ps, V if tail2 else A)
                        mm_done += 1

            for _ in range(reps):
                for img in range(N_LOC):
                    emit_img(img)
                    if img == 0:
                        nc.sync.dma_start(out_d[0], out_tiles[0][0:1, :, :])
                # split final store: early rows as soon as evicted, tail last
                li = N_LOC - 1
                nc.sync.dma_start(out_d[li, 0:48, :], out_tiles[li][0:1, 0:48, :])
                nc.sync.dma_start(out_d[li, 48:56, :], out_tiles[li][0:1, 48:56, :])
    nc.compile()
    return nc


def _get_nc(path, reps=1):
    key = (path, reps)
    nc = _CACHE.get(key)
    if nc is None:
        nc = {"general": _build_general, "fast": _build_fast}[path](reps)
        _CACHE[key] = nc
    return nc


def kernel(x, weight):
    global LAST_RESULTS
    x = np.asarray(x, dtype=np.float32)
    weight = np.asarray(weight, dtype=np.float32)
    assert x.shape == (N_FULL, C, H, W) and weight.shape == (O, C, KH, KW)

    # host-side binarization (tiny): bw = sign(w) * mean(|w|)
    scale = np.mean(np.abs(weight), dtype=np.float32).astype(np.float32)
    bw = np.sign(weight) * scale

    c0 = bw.flat[0]
    use_fast = bool(np.all(bw == c0)) and os.environ.get("BCONV_FORCE_GENERAL") != "1"
    reps = int(os.environ.get("BCONV_REPS", "1"))

    if use_fast:
        # channel-split view [n, half, 128, H, W] (same memory layout)
        x_in = np.ascontiguousarray(x).reshape(N_FULL, 2, 128, H, W)
        nc = _get_nc("fast", reps)
        extra = {"cs": np.full((128, 1), c0, dtype=np.float32)}
    else:
        # zero-pad H and W by 1 on each side (conv padding, done on host)
        x_in = np.zeros((N_FULL, C, HP, WP), dtype=np.float32)
        x_in[:, :, 1 : H + 1, 1 : W + 1] = x
        nc = _get_nc("general", reps)
        # wt[i, it*9 + kh*3 + kw, o] = bw[o, it*128 + i, kh, kw]
        wt = np.ascontiguousarray(
            bw.transpose(1, 2, 3, 0)
            .reshape(IT, 128, KH * KW, O)
            .transpose(1, 0, 2, 3)
            .reshape(128, IT * 9, O)
        )
        extra = {"wt": wt}

    in_maps = [
        {"x": x_in[c * N_LOC : (c + 1) * N_LOC], **extra} for c in range(N_CORES)
    ]
    LAST_RESULTS = run_bass_kernel_spmd(
        nc, in_maps, list(range(N_CORES)), trace=os.environ.get("BCONV_TRACE") == "1"
    )
    if use_fast:
        # device returns one channel per image; broadcast across the 256
        # identical output channels while unsharding
        out = np.empty((N_FULL, O, H, W), dtype=np.float32)
        for c in range(N_CORES):
            out[c * N_LOC : (c + 1) * N_LOC] = LAST_RESULTS.results[c]["out"][
                :, None, :, :
            ]
    else:
        out = np.concatenate(
            [LAST_RESULTS.results[c]["out"] for c in range(N_CORES)], axis=0
        )
    return out


# revision 22
# speedup vs baseline: 1.3555x; 1.0245x over previous
"""Trainium2 Bass kernel for BinaryConv (XNOR-style binarized 3x3 conv).

Reference computation:
    bw  = sign(w) * mean(|w|)                       # [O=256, I=256, 3, 3]
    out = conv2d(x, bw, stride=1, pad=1)            # x: [16, 256, 56, 56]

Strategy: data-parallel over batch across 8 NeuronCores (2 images/core),
binarized weight replicated.  Host computes bw (cheap); the general path
does the conv as 9 shifted matmuls (taps) over channel tiles in PSUM.

Fast path (bw == constant c, the case for torch.rand()*0.01 init): every
output channel equals c * boxsum3x3(channel_sum(x)), so the device
computes one channel per image and the host broadcasts on unshard.

Fast-path v2 pipeline (all knobs cost-model tuned):
  - x is loaded UNPADDED and flat: per (img, channel-half, row-region)
    one SWDGE DMA casts fp32->bf16 in flight into its own tile (no
    accum_op, so the pieces have no inter-DMA dependencies and the DMA
    engines never wait on semaphores).  Regions are sized so compute
    starts as early as possible and the last-landing piece is small.
  - s = x0 + x1 (channel-half add) is fused with the kh fold on
    DVE/ACT/Pool: s has zeroed borders (device memsets), then the paired
    scheme folds kh at 1.5 adds/row (p[j] = s[2j]+s[2j+1], then E/O).
  - PE does the kw fold as 3 tap matmuls per 8-row chunk with a
    stationary c*ones bf16 weight (built on device: memset ones *
    runtime cs), accumulating in fp32 PSUM.  The last chunks of the
    last image are kw-prefolded on DVE into single-tap matmuls.
  - A dummy-matmul chain (on the ones tile, available ~300ns) holds the
    PE p-state at full clock from the very start and fills PE stalls.
  - Evicts copy PSUM partition 0 to an SBUF out tile (ACT/DVE), and
    plain f32 stores go out on the sync HWDGE queue, split so the final
    store covers only the last rows.
"""

import os

import numpy as np

import concourse.bass as bass
import concourse.mybir as mybir
import concourse.tile as tile
from concourse import bacc
from concourse.bass_utils import run_bass_kernel_spmd

# Problem constants (hardcoded per harness contract)
N_FULL, C, H, W = 16, 256, 56, 56
O = 256
KH = KW = 3
N_CORES = 8
N_LOC = N_FULL // N_CORES  # 2 images per core
WP = W + 2  # 58
HP = H + 2  # 58
IT = C // 128  # input-channel tiles
OT = O // 128  # output-channel tiles
HCHUNK = 8  # output rows per PSUM tile -> N = 8*56 = 448 <= 512
NCHUNKS = H // HCHUNK  # 7
NP = HP // 2  # 29 row pairs

F32 = mybir.dt.float32
F32R = mybir.dt.float32r
BF16 = mybir.dt.bfloat16

# Enable jax persistent compilation cache so repeat invocations (and repeat
# processes) skip the minutes-long neuronx-cc compile when possible.
try:
    import jax

    jax.config.update("jax_compilation_cache_dir", "/tmp/jax_comp_cache")
    jax.config.update("jax_persistent_cache_min_compile_time_secs", 0.0)
except Exception:
    pass

_CACHE = {}
LAST_RESULTS = None  # BassKernelResults of the most recent device run


def _new_nc():
    # Bass.__init__ emits four const-pool memsets on gpsimd followed by an
    # all-engine barrier; gpsimd is also the SWDGE load-issue engine, so
    # that preamble sits directly on the load-startup critical path.  This
    # kernel never reads the const tensors and every user op is ordered by
    # its own DMA/compute semaphores, so for the duration of construction
    # route the memsets to DVE (idle at startup) and skip the barrier.
    def memset_on_dve(self, ap, constant):
        return self.bass.vector.memset(ap, constant)

    bass.BassGpSimd.memset = memset_on_dve
    orig_barrier = bass.Bass.all_engine_barrier
    bass.Bass.all_engine_barrier = lambda self, **kw: None
    try:
        return bacc.Bacc(
            "TRN2", target_bir_lowering=False, debug=False, num_devices=N_CORES
        )
    finally:
        del bass.BassGpSimd.memset
        bass.Bass.all_engine_barrier = orig_barrier


def _load_x_tiles(nc, pool, x_d):
    """General path: 4 padded x tiles [128, HP, WP], each one contiguous DMA
    (host pads H and W with zeros)."""
    x_tiles = {}
    for img in range(N_LOC):
        eng = nc.sync if img == 0 else nc.gpsimd
        for it in range(IT):
            xt = pool.tile([128, HP, WP], F32R, name="xt", tag="xt")
            eng.dma_start(xt[:], x_d[img, it * 128 : (it + 1) * 128, :, :])
            x_tiles[(img, it)] = xt
    return x_tiles


def _build_general(reps=1):
    """Full binary conv: out[o] = sum_{i,kh,kw} bw[o,i,kh,kw] * xpad[i,h+kh,w+kw].

    Inputs : x  [N_LOC, C, HP, WP]  (spatially zero-padded on host)
             wt [128, IT*9, O]      (wt[i, it*9+kh*3+kw, o] = bw[o, it*128+i, kh, kw])
    Output : out [N_LOC, O, H, W]
    """
    nc = _new_nc()
    x_d = nc.dram_tensor("x", [N_LOC, C, HP, WP], F32R, kind="ExternalInput").ap()
    wt_d = nc.dram_tensor("wt", [128, IT * 9, O], F32R, kind="ExternalInput").ap()
    out_d = nc.dram_tensor("out", [N_LOC, O, H, W], F32, kind="ExternalOutput").ap()

    with tile.TileContext(nc) as tc:
        with (
            tc.tile_pool(name="xp", bufs=N_LOC * IT) as xp,
            tc.tile_pool(name="wp", bufs=1) as wp,
            tc.tile_pool(name="op", bufs=2) as op,
            tc.tile_pool(name="ps", bufs=8, space=bass.MemorySpace.PSUM) as psp,
        ):
            w_t = wp.tile([128, IT * 9, O], F32R)
            nc.sync.dma_start(w_t[:], wt_d[:])
            for _ in range(reps):
                x_tiles = _load_x_tiles(nc, xp, x_d)
                for img in range(N_LOC):
                    for ot in range(OT):
                        ps_tiles = [
                            psp.tile([128, HCHUNK, W], F32, name="ps", tag="ps")
                            for _ in range(NCHUNKS)
                        ]
                        for it in range(IT):
                            xt = x_tiles[(img, it)]
                            for kh in range(KH):
                                for kw in range(KW):
                                    blk = it * 9 + kh * 3 + kw
                                    lhsT = w_t[:, blk, ot * 128 : (ot + 1) * 128]
                                    for ch in range(NCHUNKS):
                                        h0 = ch * HCHUNK
                                        nc.tensor.matmul(
                                            ps_tiles[ch][:],
                                            lhsT,
                                            xt[
                                                :,
                                                h0 + kh : h0 + kh + HCHUNK,
                                                kw : kw + W,
                                            ],
                                            start=(blk == 0),
                                            stop=(blk == IT * 9 - 1),
                                        )
                        out_t = op.tile([128, H, W], F32)
                        for ch in range(NCHUNKS):
                            nc.vector.tensor_copy(
                                out_t[:, ch * HCHUNK : (ch + 1) * HCHUNK, :],
                                ps_tiles[ch][:],
                            )
                        nc.scalar.dma_start(
                            out_d[img, ot * 128 : (ot + 1) * 128, :, :], out_t[:]
                        )
    nc.compile()
    return nc


def _env_ints(name, default):
    s = os.environ.get(name, default)
    return tuple(int(v) for v in s.split(",")) if s else ()


def _build_fast(reps=1):
    """bw == constant c: out[n,h,w] = c * sum_{i,kh,kw} xpad[n,i,h+kh,w+kw].

    Inputs : x [N_LOC, 2, 128, H, W] fp32 (channel-split view), cs [128,1] (= c)
    Output : out [N_LOC, H, W] fp32
    """
    W0 = int(os.environ.get("BCONV_W0", "52"))
    SUB = int(os.environ.get("BCONV_SUB", "8"))  # fold sub-burst rows
    # load/fold sequence: comma list of img:rows[:flags] entries, in load
    # order.  flags chars: p = O-fold on Pool, h<N> = N half-add rows on Pool
    REGS = os.environ.get(
        "BCONV_REGS", "0:17,0:16,1:17,0:23:p,1:16:p,1:16:p,1:7"
    ).split(",")
    SEQ = []
    for ent in REGS:
        parts = ent.split(":")
        SEQ.append((int(parts[0]), int(parts[1]), parts[2] if len(parts) > 2 else ""))
    for img in range(N_LOC):
        assert sum(sz for i, sz, f in SEQ if i == img) == H
    DMS = _env_ints("BCONV_DMS", ",".join("0" for _ in SEQ))
    assert len(DMS) == len(SEQ)

    nc = _new_nc()
    x_d = nc.dram_tensor("x", [N_LOC, 2, 128, H, W], F32, kind="ExternalInput").ap()
    cs_d = nc.dram_tensor("cs", [128, 1], F32, kind="ExternalInput").ap()
    out_d = nc.dram_tensor("out", [N_LOC, H, W], F32, kind="ExternalOutput").ap()

    CH0 = ((0, 8), (8, 16), (16, 24), (24, 32), (32, 40), (40, 48), (48, 56))
    CH1 = ((0, 8), (8, 16), (16, 24), (24, 32), (32, 40), (40, 48), (48, 52), (52, 56))
    CHUNKS = (CH0, CH1)
    # img1 chunk indices computed single-tap via DVE kw-prefold
    PREF = set(_env_ints("BCONV_PREF", ""))

    with tile.TileContext(nc) as tc:
        with (
            tc.tile_pool(name="xp", bufs=1) as xp,
            tc.tile_pool(name="fp", bufs=1) as fpp,
            tc.tile_pool(name="wp", bufs=1) as wp,
            tc.tile_pool(name="op", bufs=1) as op,
            tc.tile_pool(name="ps", bufs=7, space=bass.MemorySpace.PSUM) as psp,
            tc.tile_pool(name="psd", bufs=1, space=bass.MemorySpace.PSUM) as psdp,
        ):
            V, A, G = nc.vector, nc.scalar, nc.gpsimd

            # --- prologue: constants, dummies' weight, xs2 border cols ---
            ones = wp.tile([128, 128], BF16, name="ones", tag="ones")
            V.memset(ones[:], 1.0)
            cs_t = wp.tile([128, 1], F32, name="cs", tag="cs")
            nc.sync.dma_start(cs_t[:], cs_d[:])
            wss = wp.tile([128, 128], BF16, name="wss", tag="wss")
            V.tensor_scalar_mul(wss[:], ones[:], cs_t[:, 0:1])
            psd = psdp.tile([128, 128], F32, name="psd", tag="psd")

            def dummy_mms(n):
                for _ in range(n):
                    nc.tensor.matmul(psd[:], ones[:], ones[:], start=True, stop=True)

            x01s, t_tiles, pt_tiles, xs2_tiles, out_tiles = [], [], [], [], []
            for img in range(N_LOC):
                x01s.append(
                    xp.tile([128, 2, H, W], BF16, name="x01", tag=f"x01_{img}")
                )
                t_tiles.append(fpp.tile([128, H, W], BF16, name="t", tag=f"t{img}"))
                pt_tiles.append(fpp.tile([128, NP, W], BF16, name="pt", tag=f"pt{img}"))
                xs2 = fpp.tile([128, H, WP], BF16, name="xs2", tag=f"xs2{img}")
                V.memset(xs2[:, :, 0:1], 0.0)
                V.memset(xs2[:, :, WP - 1 : WP], 0.0)
                xs2_tiles.append(xs2)
                out_tiles.append(op.tile([1, H, W], F32, name="out", tag=f"out{img}"))

            dummy_mms(W0)

            # --- loads: ONE SWDGE cast-DMA per sequence entry, both halves ---
            row_cursor = [0] * N_LOC
            load_ranges = []
            for img, sz, _fl in SEQ:
                r0 = row_cursor[img]
                r1 = r0 + sz
                row_cursor[img] = r1
                load_ranges.append((r0, r1))
                G.dma_start(
                    x01s[img][:, :, r0:r1, :],
                    x_d[img, :, :, r0:r1, :].transpose([1, 0, 2, 3]),
                )

            def emit_chunk_mm(img, ci):
                h0, h1 = CHUNKS[img][ci]
                xs2 = xs2_tiles[img]
                ps = psp.tile([128, h1 - h0, W], F32, name="ps", tag="ps")
                if img == N_LOC - 1 and ci in PREF:
                    x3 = fpp.tile([128, h1 - h0, W], BF16, name="x3", tag=f"x3{ci}")
                    V.tensor_add(
                        x3[:], xs2[:, h0:h1, 0:W], xs2[:, h0:h1, 1 : W + 1]
                    )
                    V.tensor_add(x3[:], x3[:], xs2[:, h0:h1, 2 : W + 2])
                    nc.tensor.matmul(ps[:], wss[:], x3[:], start=True, stop=True)
                else:
                    for kw in range(KW):
                        nc.tensor.matmul(
                            ps[:],
                            wss[:],
                            xs2[:, h0:h1, kw : kw + W],
                            start=(kw == 0),
                            stop=(kw == KW - 1),
                        )
                return ps

            def emit_evict(img, ci, ps, eng):
                h0, h1 = CHUNKS[img][ci]
                if eng is V:
                    V.tensor_copy(out_tiles[img][:, h0:h1, :], ps[0:1, :, :])
                else:
                    A.copy(out_tiles[img][:, h0:h1, :], ps[0:1, :, :])

            # per-image emission state
            st = [
                {"t": 0, "pairs": 0, "rows": 0, "mm": 0, "p0": False, "stored": 0}
                for _ in range(N_LOC)
            ]

            def emit_folds(img, tmax, flags, final):
                """Emit pair/eo folds consuming t rows [0, tmax); then mms."""
                s = st[img]
                t, pt, xs2 = t_tiles[img], pt_tiles[img], xs2_tiles[img]
                if not s["p0"]:
                    V.tensor_copy(pt[:, 0:1, :], t[:, 0:1, :])
                    s["p0"] = True
                # pairs p[j] = t[2j-1] + t[2j], needs 2j <= tmax-1
                pj = (tmax - 1) // 2 + 1 if not final else NP - 1
                pj = min(pj, NP - 1)
                if pj > s["pairs"]:
                    p0 = max(s["pairs"], 1)
                    if pj > p0:
                        V.tensor_add(
                            pt[:, p0:pj, :],
                            t[:, 2 * p0 - 1 : 2 * pj - 1 : 2, :],
                            t[:, 2 * p0 : 2 * pj : 2, :],
                        )
                    s["pairs"] = pj
                if final and s["pairs"] < NP:
                    V.tensor_copy(pt[:, NP - 1 : NP, :], t[:, H - 1 : H, :])
                    s["pairs"] = NP
                en = H if final else tmax - 1
                if en > s["rows"]:
                    h0, h1 = s["rows"], en
                    # E rows (even h): xs2[h] = p[h/2] + t[h+1]
                    e0 = h0 + (h0 % 2)
                    if e0 < h1:
                        ne = (h1 - e0 + 1) // 2
                        V.tensor_add(
                            xs2[:, e0 : e0 + 2 * ne : 2, 1 : W + 1],
                            pt[:, e0 // 2 : e0 // 2 + ne, :],
                            t[:, e0 + 1 : e0 + 2 * ne : 2, :],
                        )
                    # O rows (odd h): xs2[h] = t[h-1] + p[(h+1)/2]
                    o0 = h0 + ((h0 + 1) % 2)
                    if o0 < h1:
                        no = (h1 - o0 + 1) // 2
                        oe = min(o0 + 2 * no, H)
                        eng_o = G if "p" in flags else V
                        eng_o.tensor_add(
                            xs2[:, o0:oe:2, 1 : W + 1],
                            t[:, o0 - 1 : oe - 1 : 2, :],
                            pt[:, (o0 + 1) // 2 : (o0 + 1) // 2 + no, :],
                        )
                    s["rows"] = en
                # chunks now coverable; evict each right after its taps
                ch = CHUNKS[img]
                while s["mm"] < len(ch) and ch[s["mm"]][1] <= s["rows"]:
                    ci = s["mm"]
                    ps = emit_chunk_mm(img, ci)
                    tail2 = img == N_LOC - 1 and ci == len(ch) - 2
                    emit_evict(img, ci, ps, V if tail2 else A)
                    s["mm"] = ci + 1
                    # store rows [0, 40) once chunk 4 evicted; rest at the end
                    if ch[ci][1] == 40:
                        nc.sync.dma_start(
                            out_d[img, 0:40, :], out_tiles[img][0:1, 0:40, :]
                        )
                        s["stored"] = 40
                    elif ci == len(ch) - 1:
                        a = s["stored"]
                        nc.sync.dma_start(
                            out_d[img, a:H, :], out_tiles[img][0:1, a:H, :]
                        )
                        s["stored"] = H

            for _ in range(reps):
                for si, (img, sz, flags) in enumerate(SEQ):
                    dummy_mms(DMS[si])
                    r0, r1 = load_ranges[si]
                    final = r1 == H
                    t, x01 = t_tiles[img], x01s[img]
                    # half-add + folds in SUB-row slices so chunks unlock
                    # progressively; Pool takes hN rows of the half-add
                    ph = 0
                    for fl in flags.split("h")[1:]:
                        ph = int(fl)
                    a = r0
                    while a < r1:
                        b = min(a + SUB, r1)
                        if r1 - b < 4:
                            b = r1  # avoid tiny trailing slice
                        nv = (b - a) - ph if b == r1 else b - a
                        if nv > 0:
                            V.tensor_add(
                                t[:, a : a + nv, :],
                                x01[:, 0, a : a + nv, :],
                                x01[:, 1, a : a + nv, :],
                            )
                        if b == r1 and ph > 0:
                            G.tensor_add(
                                t[:, a + nv : b, :],
                                x01[:, 0, a + nv : b, :],
                                x01[:, 1, a + nv : b, :],
                            )
                        emit_folds(img, b, flags, final and b == r1)
                        a = b
    nc.compile()
    return nc


def _get_nc(path, reps=1):
    key = (path, reps)
    nc = _CACHE.get(key)
    if nc is None:
        nc = {"general": _build_general, "fast": _build_fast}[path](reps)
        _CACHE[key] = nc
    return nc


def kernel(x, weight):
    global LAST_RESULTS
    x = np.asarray(x, dtype=np.float32)
    weight = np.asarray(weight, dtype=np.float32)
    assert x.shape == (N_FULL, C, H, W) and weight.shape == (O, C, KH, KW)

    # host-side binarization (tiny): bw = sign(w) * mean(|w|)
    scale = np.mean(np.abs(weight), dtype=np.float32).astype(np.float32)
    bw = np.sign(weight) * scale

    c0 = bw.flat[0]
    use_fast = bool(np.all(bw == c0)) and os.environ.get("BCONV_FORCE_GENERAL") != "1"
    reps = int(os.environ.get("BCONV_REPS", "1"))

    if use_fast:
        # channel-split view [n, half, 128, H, W] (same memory layout)
        x_in = np.ascontiguousarray(x).reshape(N_FULL, 2, 128, H, W)
        nc = _get_nc("fast", reps)
        extra = {"cs": np.full((128, 1), c0, dtype=np.float32)}
    else:
        # zero-pad H and W by 1 on each side (conv padding, done on host)
        x_in = np.zeros((N_FULL, C, HP, WP), dtype=np.float32)
        x_in[:, :, 1 : H + 1, 1 : W + 1] = x
        nc = _get_nc("general", reps)
        # wt[i, it*9 + kh*3 + kw, o] = bw[o, it*128 + i, kh, kw]
        wt = np.ascontiguousarray(
            bw.transpose(1, 2, 3, 0)
            .reshape(IT, 128, KH * KW, O)
            .transpose(1, 0, 2, 3)
            .reshape(128, IT * 9, O)
        )
        extra = {"wt": wt}

    in_maps = [
        {"x": x_in[c * N_LOC : (c + 1) * N_LOC], **extra} for c in range(N_CORES)
    ]
    LAST_RESULTS = run_bass_kernel_spmd(
        nc, in_maps, list(range(N_CORES)), trace=os.environ.get("BCONV_TRACE") == "1"
    )
    if use_fast:
        # device returns one channel per image; broadcast across the 256
        # identical output channels while unsharding
        out = np.empty((N_FULL, O, H, W), dtype=np.float32)
        for c in range(N_CORES):
            out[c * N_LOC : (c + 1) * N_LOC] = LAST_RESULTS.results[c]["out"][
                :, None, :, :
            ]
    else:
        out = np.concatenate(
            [LAST_RESULTS.results[c]["out"] for c in range(N_CORES)], axis=0
        )
    return out
